# revision 12
# baseline (speedup 1.0000x reference)
"""Trainium2 Bass kernel: GPT-style transformer forward pass.

Strategy: data-parallel over batch across 8 NeuronCores (B=64 -> 8 per core),
weights replicated.  All matmuls/activations in bf16 (PE bf16 = 1 cyc/row at
any moving-dim size, DVE 2x on 16-bit, half DMA/SBUF), PSUM accumulation f32.

Mean-free residual trick: tok/pos embeddings and the output columns of
Wproj/W2 (+ bproj/b2) are centered over the feature dim host-side, so the
residual stream x stays exactly zero-mean and LayerNorm needs NO mean
subtraction (logits are invariant: every consumer of x is a LayerNorm).
LN reduces to rstd = (E[x^2]+eps)^-1/2 via one ones-matmul of x^2 and
h = x * rstd.  Q/K biases are applied on the PSUM->SBUF copy (per-partition
bias columns) instead of via rank-1 matmuls.

Attention: transposed scores S_T[s,t] per (b,h), multiplicative causal mask
after exp, token-major PV in bf16 (moving dim 66 at full bf16 rate) with an
appended ones-column in V so softmax denominators land as a per-partition
column, and bf16 PE transposes carry att back to feature-major.
"""

import os
import sys

for _p in ("/opt/trn_rl_repo",):
    if _p not in sys.path and os.path.isdir(_p):
        sys.path.insert(0, _p)

import numpy as np
import ml_dtypes

import concourse.bass as bass
import concourse.mybir as mybir
import concourse.tile as tile
from concourse import bacc
from concourse.bass_utils import run_bass_kernel_spmd

F32 = mybir.dt.float32
BF16 = mybir.dt.bfloat16
AF = mybir.ActivationFunctionType
OP = mybir.AluOpType

NPBF = ml_dtypes.bfloat16

V, D, H, HD, L, T, B = 65, 384, 6, 64, 6, 256, 64
NCORES = 8
BL = B // NCORES          # batch elements per core
NTOK = BL * T             # tokens per core
DFF = 4 * D               # 1536
EPS = 1e-3
KC = D // 128             # 3 contraction chunks of 128
MT = D // 128             # 3 output feature tiles
FT = DFF // 128           # 12 ffn tiles
HD1 = HD + 2              # V columns incl ones col (64) + pad (65)
MDT = BF16

# stat matmul uses 1/512 (exact in bf16); corrections folded into Ln/Exp:
#   msq = sum(x^2)/512 = var * D/512
#   rstd = exp(-0.5*ln(msq + EPS*D/512) + 0.5*ln(D/512))
EPS_EFF = EPS * D / 512.0
RSTD_BIAS = 0.5 * float(np.log(D / 512.0))

# cst layout: cols 0:512 ones, col 512 unused, cols 513:641 identity 128x128,
# cols 641:769 = 1/512 block (stats matmul lhsT)
CST_W = 513 + 128 + 128


def _patch_act_tables():
    """Steer the activation-table picker to natural_log_exp_and_others for
    Exp and Ln, so this kernel's ACT stream never switches table sets."""
    if getattr(bacc, "_act_tables_patched", False):
        return
    real = bacc.get_activation_tables

    def patched(arch):
        t = real(arch)
        exp = mybir.ActivationFunctionType.Exp
        ln = mybir.ActivationFunctionType.Ln
        out = {}
        for name, fns in t.items():
            if name != "natural_log_exp_and_others":
                fns = fns - {exp, ln}
            out[name] = fns
        return out

    bacc.get_activation_tables = patched
    bacc._act_tables_patched = True


class _MM:
    """matmul emitter with explicit accumulation-chain boundaries."""

    def __init__(self, nc):
        self.nc = nc

    def __call__(self, out, lhsT, rhs, first=True, last=True, tile_position=None):
        self.nc.tensor.matmul(
            out, lhsT, rhs, start=first, stop=last, tile_position=tile_position,
        )


def build_program(n_layers=L, n_b=BL, n_heads=H):
    _patch_act_tables()
    assert n_b % 2 == 0 or n_b == 1
    ntok = n_b * T
    npair = max(1, n_b // 2)
    PW = 512 if n_b > 1 else 256      # tokens per pair-chunk
    nc = bacc.Bacc("TRN2", target_bir_lowering=False, debug=False)

    # ---------------- DRAM I/O ----------------
    hot_d = nc.dram_tensor("hotT", [V, ntok], MDT, kind="ExternalInput").ap()
    temb_d = nc.dram_tensor("temb", [V, D], MDT, kind="ExternalInput").ap()
    pos_d = nc.dram_tensor("posT", [128, KC, 512], MDT, kind="ExternalInput").ap()
    mask_d = nc.dram_tensor("mask", [128, 512], MDT, kind="ExternalInput").ap()
    wqkv_d = nc.dram_tensor("wqkv", [n_layers, 128, 3, KC, D], MDT, kind="ExternalInput").ap()
    wproj_d = nc.dram_tensor("wproj", [n_layers, 128, KC, D], MDT, kind="ExternalInput").ap()
    w1_d = nc.dram_tensor("w1", [n_layers, 128, KC, DFF], MDT, kind="ExternalInput").ap()
    w2_d = nc.dram_tensor("w2", [n_layers, 128, FT, D], MDT, kind="ExternalInput").ap()
    vbias_d = nc.dram_tensor("vbias", [n_layers, 128, D], F32, kind="ExternalInput").ap()
    biasc_d = nc.dram_tensor("biasc", [n_layers, 128, MT + FT + MT], F32, kind="ExternalInput").ap()
    biasr_d = nc.dram_tensor("biasr", [n_layers, 128, 2 * MT], F32, kind="ExternalInput").ap()
    whead_d = nc.dram_tensor("whead", [128, KC, V], MDT, kind="ExternalInput").ap()
    bhead_d = nc.dram_tensor("bheadc", [V, 1], F32, kind="ExternalInput").ap()
    lnc_d = nc.dram_tensor("lnc", [128, 2], F32, kind="ExternalInput").ap()
    cst_d = nc.dram_tensor("cst", [128, CST_W], MDT, kind="ExternalInput").ap()
    out_d = nc.dram_tensor("logitsT", [n_b, V, T], F32, kind="ExternalOutput").ap()

    from contextlib import ExitStack

    with tile.TileContext(nc) as tc, \
         nc.allow_low_precision(reason="bf16 matmul operand production"), \
         ExitStack() as ctx:
        ep = ctx.enter_context

        # ---------------- pools ----------------
        cpool = ep(tc.tile_pool(name="consts", bufs=1))
        xpool = ep(tc.tile_pool(name="x", bufs=1))
        wpool_qkv = ep(tc.tile_pool(name="wqkv", bufs=1))
        wpool_proj = ep(tc.tile_pool(name="wproj", bufs=1))
        wpool_1 = ep(tc.tile_pool(name="w1", bufs=1))
        wpool_2 = ep(tc.tile_pool(name="w2", bufs=1))
        wpool_b = ep(tc.tile_pool(name="wbias", bufs=1))
        hpool = ep(tc.tile_pool(name="h", bufs=3))
        xsqpool = ep(tc.tile_pool(name="xsq", bufs=2))
        hotpool = ep(tc.tile_pool(name="hot", bufs=2))
        qpool = ep(tc.tile_pool(name="q", bufs=2))
        kpool = ep(tc.tile_pool(name="k", bufs=2))
        vpool = ep(tc.tile_pool(name="v", bufs=1))
        upool = ep(tc.tile_pool(name="u", bufs=4))
        atmpool = ep(tc.tile_pool(name="atm", bufs=2))
        attpool = ep(tc.tile_pool(name="att", bufs=1))
        h1pool = ep(tc.tile_pool(name="h1", bufs=1))
        lgpool = ep(tc.tile_pool(name="lg", bufs=1))
        stpool = ep(tc.tile_pool(name="st", bufs=6))
        rdpool = ep(tc.tile_pool(name="rd", bufs=4))

        pbig = ep(tc.tile_pool(name="pbig", bufs=4, space="PSUM"))
        ppv = ep(tc.tile_pool(name="ppv", bufs=2, space="PSUM"))
        pstat = ep(tc.tile_pool(name="pstat", bufs=2, space="PSUM"))

        mm = _MM(nc)

        # ---------------- constants ----------------
        cst = cpool.tile([128, CST_W], MDT, name="cst_c")
        nc.sync.dma_start(out=cst[:, :], in_=cst_d[:, :])
        ones = cst[:, 0:512]
        ident = cst[:, 513:641]
        inv512 = cst[:, 641:769]
        mask = cpool.tile([128, 512], MDT, name="mask_c")
        nc.sync.dma_start(out=mask[:, :], in_=mask_d[:, :])
        whead = cpool.tile([128, KC, V], MDT, name="whead_c")
        nc.sync.dma_start(out=whead[:, :, :], in_=whead_d[:, :, :])
        bhead = cpool.tile([V, 1], F32, name="bhead_c")
        nc.sync.dma_start(out=bhead[:, :], in_=bhead_d[:, :])
        lnc = cpool.tile([128, 2], F32, name="lnc_c")
        nc.sync.dma_start(out=lnc[:, :], in_=lnc_d[:, :])

        x = xpool.tile([128, KC, ntok], MDT, name="x_resid")

        nbp = PW // T
        v_tiles = [vpool.tile([128, 2 * nbp, n_heads, HD1], MDT, name=f"v_pp{i}")
                   for i in range(2)]
        for vt in v_tiles:
            for tb in range(2 * nbp):
                nc.vector.tensor_copy(vt[:, tb, :, HD:HD1],
                                      ones[:, 0:2 * n_heads].rearrange(
                                          "p (h c) -> p h c", h=n_heads))

        # ---------------- LN (mean-free): sq -> mm+rstd -> finish ----------------
        def ln_sq(p):
            """x^2 on ACT; emitted early so the PE stat matmuls never stall"""
            pc = slice(p * PW, p * PW + PW)
            xsq = xsqpool.tile([128, KC, PW], MDT, tag="xsq")
            nc.scalar.activation(xsq[:, :, :], x[:, :, pc], AF.Square)
            return (p, xsq)

        def ln_mmrstd(tok):
            """ones-matmul partition reduction -> Ln -> Exp -> rstd"""
            p, xsq = tok
            msqb = pstat.tile([128, PW], F32, tag="pstat")
            for c in range(KC):
                mm(msqb[:, :], inv512[:, :], xsq[:, c, :],
                   first=(c == 0), last=(c == KC - 1))
            lnv = stpool.tile([128, PW], F32, tag="st")
            nc.scalar.activation(lnv[:, :], msqb[:, :], AF.Ln, bias=lnc[:, 0:1])
            rstdb = stpool.tile([128, PW], MDT, tag="st")
            nc.scalar.activation(rstdb[:, :], lnv[:, :], AF.Exp,
                                 scale=-0.5, bias=lnc[:, 1:2])
            return (p, rstdb)

        def ln_stats(p):
            return ln_mmrstd(ln_sq(p))

        def ln_finish(tok):
            """apply -> h (pure DVE; no PE work)"""
            p, rstdb = tok
            pc = slice(p * PW, p * PW + PW)
            h = hpool.tile([128, KC, PW], MDT, tag="h")
            for c in range(KC):
                nc.vector.tensor_tensor(h[:, c, :], x[:, c, pc], rstdb[:, :], op=OP.mult)
            return h

        # ---------------- embedding (scratch tiles borrow other pools) ----------------
        pos = attpool.tile([128, KC, 512], MDT, tag="att")
        nc.sync.dma_start(out=pos[:, :, :], in_=pos_d[:, :, :])
        temb = stpool.tile([V, 384], MDT, tag="st")
        nc.sync.dma_start(out=temb[:, :], in_=temb_d[:, :])
        emb_sq = {}
        for ch in range(ntok // PW):
            cs = slice(ch * PW, ch * PW + PW)
            hot = hotpool.tile([V, PW], MDT, tag="hot")
            nc.sync.dma_start(out=hot[:, :], in_=hot_d[:, cs])
            for c in range(KC):
                ps = pbig.tile([128, 512], F32, tag="pbig")
                mm(ps[:, 0:PW], temb[0:V, c * 128:(c + 1) * 128], hot[0:V, :])
                nc.vector.tensor_tensor(x[:, c, cs], ps[:, 0:PW], pos[:, c, 0:PW], op=OP.add)
            if ch < 2:
                emb_sq[ch] = ln_sq(ch)   # overlap Square with later emb chunks

        def run(mids):
            out = []
            for f in mids:
                out.append(f())
            return out

        # ---------------- phase B: qkv + attention + proj for a pair ----------------
        def emit_B(p, h, wqkv, wproj, vbias, biasc, biasr, mid_a=(), mid_b=()):
            pc = slice(p * PW, p * PW + PW)
            nb_in_p = PW // T
            q_t = qpool.tile([128, MT, PW], MDT, tag="q")
            k_t = kpool.tile([128, MT, PW], MDT, tag="k")
            for mat, dst in ((0, q_t), (1, k_t)):
                for mt in range(MT):
                    ps = pbig.tile([128, 512], F32, tag="pbig")
                    for kc in range(KC):
                        mm(ps[:, 0:PW], wqkv[:, mat, kc, mt * 128:(mt + 1) * 128],
                           h[:, kc, :], first=(kc == 0), last=(kc == KC - 1))
                    nc.vector.tensor_scalar(
                        dst[:, mt, :], ps[:, 0:PW],
                        biasr[:, mat * MT + mt:mat * MT + mt + 1], None, op0=OP.add)
            v_t = v_tiles[p % 2]
            for tb in range(2 * nb_in_p):
                vps = pbig.tile([128, 512], F32, tag="pbig")
                for kc in range(KC):
                    mm(vps[:, 0:D], h[:, kc, tb * 128:(tb + 1) * 128],
                       wqkv[:, 2, kc, :], first=(kc == 0), last=(kc == KC - 1))
                nc.vector.tensor_tensor(
                    v_t[:, tb, :, 0:HD],
                    vps[:, 0:D].rearrange("p (h d) -> p h d", h=n_heads),
                    vbias[:, :].rearrange("p (h d) -> p h d", h=n_heads),
                    op=OP.add)
            mids_out = run(mid_a)
            atms = []
            for bi in range(nb_in_p):
                boff = bi * T
                if bi == 1:
                    mids_out += run(mid_b)
                us = [None] * n_heads
                pv0 = ppv.tile([128, n_heads, HD1], F32, tag="ppv")
                pv1 = ppv.tile([128, n_heads, HD1], F32, tag="ppv")

                def emit_S(hh):
                    hp = 64 * (hh % 2)
                    hc = hh // 2
                    sps = pbig.tile([128, 512], F32, tag="pbig")
                    mm(sps[:, 0:256], k_t[hp:hp + HD, hc, boff:boff + 128],
                       q_t[hp:hp + HD, hc, boff:boff + T])
                    mm(sps[:, 384:512], k_t[hp:hp + HD, hc, boff + 128:boff + 256],
                       q_t[hp:hp + HD, hc, boff + 128:boff + 256])
                    # cols 256:384 (keys 128-255 x queries 0-127) are fully
                    # masked and never read by PV: skip exp there entirely;
                    # cols 128:256 (keys 0-127 x queries 128-255) are fully
                    # valid: no mask multiply needed.
                    u_t = upool.tile([128, 512], MDT, tag="u")
                    nc.scalar.activation(u_t[:, 0:256], sps[:, 0:256], AF.Exp)
                    nc.scalar.activation(u_t[:, 384:512], sps[:, 384:512], AF.Exp)
                    nc.vector.tensor_tensor(u_t[:, 0:128], u_t[:, 0:128],
                                            mask[:, 0:128], op=OP.mult)
                    nc.vector.tensor_tensor(u_t[:, 384:512], u_t[:, 384:512],
                                            mask[:, 384:512], op=OP.mult)
                    us[hh] = u_t

                def emit_PV(hh):
                    u_t = us[hh]
                    mm(pv0[:, hh, :], u_t[:, 0:128], v_t[:, 2 * bi, hh, :])
                    mm(pv1[:, hh, :], u_t[:, 128:256], v_t[:, 2 * bi, hh, :],
                       first=True, last=False)
                    mm(pv1[:, hh, :], u_t[:, 384:512], v_t[:, 2 * bi + 1, hh, :],
                       first=False, last=True)

                # window the S/PV interleave so only ~3 U tiles are live
                emit_S(0); emit_S(1); emit_S(2)
                emit_PV(0); emit_S(3)
                emit_PV(1); emit_S(4)
                emit_PV(2); emit_S(5)
                emit_PV(3); emit_PV(4); emit_PV(5)
                atm = atmpool.tile([128, 2, n_heads * HD], MDT, tag="atm")
                for tb, pv in ((0, pv0), (1, pv1)):
                    rden = rdpool.tile([128, n_heads], F32, tag="rd")
                    nc.vector.reciprocal(rden[:, :], pv[:, :, HD])
                    nc.vector.tensor_tensor(
                        atm[:, tb, :].rearrange("p (h d) -> p h d", h=n_heads),
                        pv[:, :, 0:HD],
                        rden[:, :, None].broadcast_to([128, n_heads, HD]),
                        op=OP.mult)
                atms.append(atm)
            att_t = attpool.tile([128, KC, PW], MDT, tag="att")
            for c in range(KC):
                tps = pbig.tile([128, 512], MDT, tag="pbig")
                for bi in range(nb_in_p):
                    for tb in range(2):
                        col = (bi * 2 + tb) * 128
                        nc.tensor.transpose(
                            tps[:, col:col + 128],
                            atms[bi][:, tb, c * 128:(c + 1) * 128],
                            ident[:, :])
                nc.vector.tensor_copy(att_t[:, c, :], tps[:, 0:PW])
            for mt in range(MT):
                pp = pbig.tile([128, 512], F32, tag="pbig")
                for kc in range(KC):
                    mm(pp[:, 0:PW], wproj[:, kc, mt * 128:(mt + 1) * 128],
                       att_t[:, kc, :], first=(kc == 0), last=(kc == KC - 1))
                nc.vector.scalar_tensor_tensor(
                    x[:, mt, pc], pp[:, 0:PW], biasc[:, mt:mt + 1], x[:, mt, pc],
                    op0=OP.add, op1=OP.add)
            return mids_out

        # ---------------- phase D: FFN for a pair ----------------
        def emit_D(p, h2, w1, w2, biasc, mid=()):
            pc = slice(p * PW, p * PW + PW)
            h1_t = h1pool.tile([128, FT, PW], MDT, tag="h1")
            for mt in range(FT):
                fps = pbig.tile([128, 512], F32, tag="pbig")
                for kc in range(KC):
                    mm(fps[:, 0:PW], w1[:, kc, mt * 128:(mt + 1) * 128],
                       h2[:, kc, :], first=(kc == 0), last=(kc == KC - 1))
                nc.scalar.activation(h1_t[:, mt, :], fps[:, 0:PW], AF.Relu,
                                     bias=biasc[:, MT + mt:MT + mt + 1])
            mids_out = run(mid)
            for mt in range(MT):
                fp2 = pbig.tile([128, 512], F32, tag="pbig")
                for kc in range(FT):
                    mm(fp2[:, 0:PW], w2[:, kc, mt * 128:(mt + 1) * 128],
                       h1_t[:, kc, :], first=(kc == 0), last=(kc == FT - 1))
                nc.vector.scalar_tensor_tensor(
                    x[:, mt, pc], fp2[:, 0:PW],
                    biasc[:, MT + FT + mt:MT + FT + mt + 1],
                    x[:, mt, pc], op0=OP.add, op1=OP.add)
            return mids_out

        # ---------------- layers: software-pipelined emission ----------------
        carry = {}
        for l in range(n_layers):
            wqkv = wpool_qkv.tile([128, 3, KC, D], MDT, tag="wqkv")
            nc.sync.dma_start(out=wqkv[:, :, :, :], in_=wqkv_d[l])
            wproj = wpool_proj.tile([128, KC, D], MDT, tag="wproj")
            nc.sync.dma_start(out=wproj[:, :, :], in_=wproj_d[l])
            w1 = wpool_1.tile([128, KC, DFF], MDT, tag="w1")
            nc.sync.dma_start(out=w1[:, :, :], in_=w1_d[l])
            w2 = wpool_2.tile([128, FT, D], MDT, tag="w2")
            nc.sync.dma_start(out=w2[:, :, :], in_=w2_d[l])
            vbias = wpool_b.tile([128, D], F32, tag="vbias")
            nc.sync.dma_start(out=vbias[:, :], in_=vbias_d[l])
            biasc = wpool_b.tile([128, MT + FT + MT], F32, tag="biasc")
            nc.sync.dma_start(out=biasc[:, :], in_=biasc_d[l])
            biasr = wpool_b.tile([128, 2 * MT], F32, tag="biasr")
            nc.sync.dma_start(out=biasr[:, :], in_=biasr_d[l])

            Bf = lambda p, h, **kw: emit_B(p, h, wqkv, wproj, vbias, biasc, biasr, **kw)
            Dp = lambda p, h2, **kw: emit_D(p, h2, w1, w2, biasc, **kw)

            if npair == 4:
                if l == 0:
                    s0 = ln_mmrstd(emb_sq[0])
                    h0 = ln_finish(s0)
                    s1 = ln_mmrstd(emb_sq[1])
                else:
                    h0, s1 = carry["h0"], carry["s1"]
                # naming: sN = LN1 stats token pair N; scN = LN2 stats token
                (h1,) = Bf(0, h0, mid_a=[lambda: ln_finish(s1)])
                q0 = ln_sq(0)                   # LN2 p0 (x p0 updated by B0)
                h2_0, s2 = Bf(1, h1, mid_a=[lambda: ln_finish(ln_mmrstd(q0)),
                                            lambda: ln_mmrstd(ln_sq(2))])
                (g2,) = Dp(0, h2_0, mid=[lambda: ln_finish(s2)])
                q1 = ln_sq(1)                   # LN2 p1 (x p1 updated by B1)
                h2_1, s3 = Bf(2, g2, mid_a=[lambda: ln_finish(ln_mmrstd(q1)),
                                            lambda: ln_mmrstd(ln_sq(3))])
                (g3,) = Dp(1, h2_1, mid=[lambda: ln_finish(s3)])
                q2 = ln_sq(2)                   # LN2 p2 (x p2 updated by B2)
                (h2_2,) = Bf(3, g3, mid_a=[lambda: ln_finish(ln_mmrstd(q2))])
                last = (l == n_layers - 1)
                q3 = ln_sq(3)                   # LN2 p3 (x p3 updated by B3)
                if not last:
                    q0n = ln_sq(0)              # LN1-next p0 (x p0 upd by D0)
                    sc3, s0n = Dp(2, h2_2, mid=[lambda: ln_mmrstd(q3),
                                                lambda: ln_mmrstd(q0n)])
                    h2_3 = ln_finish(sc3)
                    q1n = ln_sq(1)              # LN1-next p1 (x p1 upd by D1)
                    h0n, s1n = Dp(3, h2_3, mid=[lambda: ln_finish(s0n),
                                                lambda: ln_mmrstd(q1n)])
                    carry = {"h0": h0n, "s1": s1n}
                else:
                    qf0 = ln_sq(0)              # final LN p0 (x p0 upd by D0)
                    sc3, sf0 = Dp(2, h2_2, mid=[lambda: ln_mmrstd(q3),
                                                lambda: ln_mmrstd(qf0)])
                    h2_3 = ln_finish(sc3)
                    qf1 = ln_sq(1)
                    hf0, sf1 = Dp(3, h2_3, mid=[lambda: ln_finish(sf0),
                                                lambda: ln_mmrstd(qf1)])
                    carry = {"hf0": hf0, "sf1": sf1}
            else:
                # simple order for small test configs
                hq = {}
                hq[0] = ln_finish(ln_stats(0))
                for p in range(1, npair):
                    hq[p] = ln_finish(ln_stats(p))
                    Bf(p - 1, hq.pop(p - 1))
                Bf(npair - 1, hq.pop(npair - 1))
                hq[0] = ln_finish(ln_stats(0))
                for p in range(1, npair):
                    hq[p] = ln_finish(ln_stats(p))
                    Dp(p - 1, hq.pop(p - 1))
                Dp(npair - 1, hq.pop(npair - 1))

        # ---------------- final LN + head ----------------
        def emit_head(p, hf):
            for bi in range(PW // T):
                b = p * (PW // T) + bi
                hps = ppv.tile([V, 256], F32, tag="ppv")
                for kc in range(KC):
                    mm(hps[:, :], whead[:, kc, :], hf[:, kc, bi * T:(bi + 1) * T],
                       first=(kc == 0), last=(kc == KC - 1))
                lg = lgpool.tile([V, T], F32, tag="lg")
                nc.vector.tensor_scalar(lg[:, :], hps[:, :], bhead[0:V, 0:1], None,
                                        op0=OP.add)
                nc.sync.dma_start(out=out_d[b], in_=lg[:, :])

        if npair == 4:
            hf, sf_next = carry["hf0"], carry["sf1"]
            for p in range(npair):
                if p + 2 < npair:
                    qn = ln_sq(p + 2)
                else:
                    qn = None
                emit_head(p, hf)
                if p + 1 < npair:
                    hf = ln_finish(sf_next)
                    if qn is not None:
                        sf_next = ln_mmrstd(qn)
        else:
            for p in range(npair):
                emit_head(p, ln_finish(ln_stats(p)))

    nc.compile()
    return nc


# ---------------------------------------------------------------------------
# host side
# ---------------------------------------------------------------------------

def prep_inputs(inputs, n_layers=L, n_b=BL, core=0):
    """Build the per-core input map (numpy) for `core`."""
    f32 = np.float32
    idx = np.asarray(inputs["idx"])
    tok_emb = np.asarray(inputs["tok_emb"], f32)
    pos_emb = np.asarray(inputs["pos_emb"], f32)
    Wq = np.asarray(inputs["Wq"], f32)
    Wk = np.asarray(inputs["Wk"], f32)
    Wv = np.asarray(inputs["Wv"], f32)
    Wproj = np.asarray(inputs["Wproj"], f32)
    bproj = np.asarray(inputs["bproj"], f32)
    W1 = np.asarray(inputs["W1"], f32)
    b1 = np.asarray(inputs["b1"], f32)
    W2 = np.asarray(inputs["W2"], f32)
    b2 = np.asarray(inputs["b2"], f32)
    ln1_g = np.asarray(inputs["ln1_g"], f32)
    ln1_b = np.asarray(inputs["ln1_b"], f32)
    ln2_g = np.asarray(inputs["ln2_g"], f32)
    ln2_b = np.asarray(inputs["ln2_b"], f32)
    lnf_g = np.asarray(inputs["lnf_g"], f32)
    lnf_b = np.asarray(inputs["lnf_b"], f32)
    Whead = np.asarray(inputs["Whead"], f32)
    bhead = np.asarray(inputs["bhead"], f32)

    ntok = n_b * T
    scale = f32(D) ** -0.5

    # mean-free residual: center embeddings + Wproj/W2 output columns
    tok_emb = tok_emb - tok_emb.mean(-1, keepdims=True)
    pos_emb = pos_emb - pos_emb.mean(-1, keepdims=True)
    Wproj = Wproj - Wproj.mean(-1, keepdims=True)
    W2 = W2 - W2.mean(-1, keepdims=True)
    bproj = bproj - bproj.mean(-1, keepdims=True)
    b2 = b2 - b2.mean(-1, keepdims=True)

    idx_c = idx[core * n_b:(core + 1) * n_b].reshape(-1)         # [ntok]
    hot = (idx_c[None, :] == np.arange(V)[:, None]).astype(NPBF)  # [V, ntok]

    posT = pos_emb.T.astype(f32)                                 # [D, T]
    posT2 = np.concatenate([posT, posT], axis=1)                 # [D, 512]
    pos_in = posT2.reshape(KC, 128, 512).transpose(1, 0, 2).astype(NPBF)

    lane = np.arange(128)
    t = np.arange(T)
    m0 = (lane[:, None] <= t[None, :]).astype(NPBF)
    m1 = ((lane[:, None] + 128) <= t[None, :]).astype(NPBF)
    mask = np.concatenate([m0, m1], axis=1)                      # [128, 512]

    def pack_w(w):  # [D_in, N] -> [128, KC_in, N]
        kin = w.shape[0] // 128
        return w.reshape(kin, 128, -1).transpose(1, 0, 2).copy()

    wqkv = np.zeros((n_layers, 128, 3, KC, D), NPBF)
    wproj = np.zeros((n_layers, 128, KC, D), NPBF)
    w1 = np.zeros((n_layers, 128, KC, DFF), NPBF)
    w2 = np.zeros((n_layers, 128, FT, D), NPBF)
    vbias = np.zeros((n_layers, 128, D), f32)
    biasc = np.zeros((n_layers, 128, MT + FT + MT), f32)
    biasr = np.zeros((n_layers, 128, 2 * MT), f32)

    for l in range(n_layers):
        # Wq[l] is [H, D, HD]; feature f = h*HD+hd -> transpose to [D, H, HD]
        wq2 = Wq[l].transpose(1, 0, 2).reshape(D, D) * scale
        wk2 = Wk[l].transpose(1, 0, 2).reshape(D, D)
        wv2 = Wv[l].transpose(1, 0, 2).reshape(D, D)
        wqkv[l, :, 0] = pack_w(wq2 * ln1_g[l][:, None])
        wqkv[l, :, 1] = pack_w(wk2 * ln1_g[l][:, None])
        wqkv[l, :, 2] = pack_w(wv2 * ln1_g[l][:, None])
        biasr[l, :, 0:MT] = (ln1_b[l] @ wq2).reshape(MT, 128).T
        biasr[l, :, MT:2 * MT] = (ln1_b[l] @ wk2).reshape(MT, 128).T
        vbias[l] = np.broadcast_to(ln1_b[l] @ wv2, (128, D))
        wproj[l] = pack_w(Wproj[l])
        w1[l] = pack_w(W1[l] * ln2_g[l][:, None])
        w2[l] = pack_w(W2[l])
        biasc[l, :, 0:MT] = bproj[l].reshape(MT, 128).T
        biasc[l, :, MT:MT + FT] = (b1[l] + ln2_b[l] @ W1[l]).reshape(FT, 128).T
        biasc[l, :, MT + FT:] = b2[l].reshape(MT, 128).T

    whead_eff = Whead * lnf_g[:, None]
    bhead_eff = (bhead + lnf_b @ Whead).astype(f32)

    cst = np.ones((128, CST_W), NPBF)
    cst[:, 513:641] = np.eye(128, dtype=NPBF)
    cst[:, 641:769] = NPBF(1.0 / 512.0)

    lnc = np.zeros((128, 2), f32)
    lnc[:, 0] = EPS_EFF
    lnc[:, 1] = RSTD_BIAS

    return {
        "cst": cst,
        "lnc": lnc,
        "hotT": hot,
        "temb": tok_emb.astype(NPBF),
        "posT": pos_in,
        "mask": mask,
        "wqkv": wqkv,
        "wproj": wproj,
        "w1": w1,
        "w2": w2,
        "vbias": vbias,
        "biasc": biasc,
        "biasr": biasr,
        "whead": pack_w(whead_eff).astype(NPBF),
        "bheadc": bhead_eff[:, None].copy(),
    }


_CACHE = {}


def get_program():
    if "nc" not in _CACHE:
        _CACHE["nc"] = build_program()
    return _CACHE["nc"]


def run_on_hw(inputs, trace=False):
    nc = get_program()
    in_maps = [prep_inputs(inputs, core=c) for c in range(NCORES)]
    res = run_bass_kernel_spmd(nc, in_maps, list(range(NCORES)), trace=trace)
    outs = []
    for c in range(NCORES):
        lt = res.results[c]["logitsT"]          # [BL, V, T]
        outs.append(lt.transpose(0, 2, 1))      # [BL, T, V]
    full = np.concatenate(outs, axis=0)         # [B, T, V]
    return full, res


def kernel(**inputs):
    out, _ = run_on_hw(inputs, trace=False)
    return out


# revision 13
# speedup vs baseline: 1.0002x; 1.0002x over previous
"""Trainium2 Bass kernel: GPT-style transformer forward pass.

Strategy: data-parallel over batch across 8 NeuronCores (B=64 -> 8 per core),
weights replicated.  All matmuls/activations in bf16 (PE bf16 = 1 cyc/row at
any moving-dim size, DVE 2x on 16-bit, half DMA/SBUF), PSUM accumulation f32.

Mean-free residual trick: tok/pos embeddings and the output columns of
Wproj/W2 (+ bproj/b2) are centered over the feature dim host-side, so the
residual stream x stays exactly zero-mean and LayerNorm needs NO mean
subtraction (logits are invariant: every consumer of x is a LayerNorm).
LN reduces to rstd = (E[x^2]+eps)^-1/2 via one ones-matmul of x^2 and
h = x * rstd.  Q/K biases are applied on the PSUM->SBUF copy (per-partition
bias columns) instead of via rank-1 matmuls.

Attention: transposed scores S_T[s,t] per (b,h), multiplicative causal mask
after exp, token-major PV in bf16 (moving dim 66 at full bf16 rate) with an
appended ones-column in V so softmax denominators land as a per-partition
column, and bf16 PE transposes carry att back to feature-major.
"""

import os
import sys

for _p in ("/opt/trn_rl_repo",):
    if _p not in sys.path and os.path.isdir(_p):
        sys.path.insert(0, _p)

import numpy as np
import ml_dtypes

import concourse.bass as bass
import concourse.mybir as mybir
import concourse.tile as tile
from concourse import bacc
from concourse.bass_utils import run_bass_kernel_spmd

F32 = mybir.dt.float32
BF16 = mybir.dt.bfloat16
AF = mybir.ActivationFunctionType
OP = mybir.AluOpType

NPBF = ml_dtypes.bfloat16

V, D, H, HD, L, T, B = 65, 384, 6, 64, 6, 256, 64
NCORES = 8
BL = B // NCORES          # batch elements per core
NTOK = BL * T             # tokens per core
DFF = 4 * D               # 1536
EPS = 1e-3
KC = D // 128             # 3 contraction chunks of 128
MT = D // 128             # 3 output feature tiles
FT = DFF // 128           # 12 ffn tiles
HD1 = HD + 2              # V columns incl ones col (64) + pad (65)
MDT = BF16

# stat matmul uses 1/512 (exact in bf16); corrections folded into Ln/Exp:
#   msq = sum(x^2)/512 = var * D/512
#   rstd = exp(-0.5*ln(msq + EPS*D/512) + 0.5*ln(D/512))
EPS_EFF = EPS * D / 512.0
RSTD_BIAS = 0.5 * float(np.log(D / 512.0))

# cst layout: cols 0:512 ones, col 512 unused, cols 513:641 identity 128x128,
# cols 641:769 = 1/512 block (stats matmul lhsT)
CST_W = 513 + 128 + 128


def _patch_act_tables():
    """Steer the activation-table picker to natural_log_exp_and_others for
    Exp and Ln, so this kernel's ACT stream never switches table sets."""
    if getattr(bacc, "_act_tables_patched", False):
        return
    real = bacc.get_activation_tables

    def patched(arch):
        t = real(arch)
        exp = mybir.ActivationFunctionType.Exp
        ln = mybir.ActivationFunctionType.Ln
        out = {}
        for name, fns in t.items():
            if name != "natural_log_exp_and_others":
                fns = fns - {exp, ln}
            out[name] = fns
        return out

    bacc.get_activation_tables = patched
    bacc._act_tables_patched = True


class _MM:
    """matmul emitter with explicit accumulation-chain boundaries."""

    def __init__(self, nc):
        self.nc = nc

    def __call__(self, out, lhsT, rhs, first=True, last=True, tile_position=None):
        self.nc.tensor.matmul(
            out, lhsT, rhs, start=first, stop=last, tile_position=tile_position,
        )


def build_program(n_layers=L, n_b=BL, n_heads=H):
    _patch_act_tables()
    assert n_b % 2 == 0 or n_b == 1
    ntok = n_b * T
    npair = max(1, n_b // 2)
    PW = 512 if n_b > 1 else 256      # tokens per pair-chunk
    nc = bacc.Bacc("TRN2", target_bir_lowering=False, debug=False)

    # ---------------- DRAM I/O ----------------
    hot_d = nc.dram_tensor("hotT", [V, ntok], MDT, kind="ExternalInput").ap()
    temb_d = nc.dram_tensor("temb", [V, D], MDT, kind="ExternalInput").ap()
    pos_d = nc.dram_tensor("posT", [128, KC, 512], MDT, kind="ExternalInput").ap()
    mask_d = nc.dram_tensor("mask", [128, 512], MDT, kind="ExternalInput").ap()
    wqkv_d = nc.dram_tensor("wqkv", [n_layers, 128, 3, KC, D], MDT, kind="ExternalInput").ap()
    wproj_d = nc.dram_tensor("wproj", [n_layers, 128, KC, D], MDT, kind="ExternalInput").ap()
    w1_d = nc.dram_tensor("w1", [n_layers, 128, KC, DFF], MDT, kind="ExternalInput").ap()
    w2_d = nc.dram_tensor("w2", [n_layers, 128, FT, D], MDT, kind="ExternalInput").ap()
    vbias_d = nc.dram_tensor("vbias", [n_layers, 128, D], F32, kind="ExternalInput").ap()
    biasc_d = nc.dram_tensor("biasc", [n_layers, 128, MT + FT + MT], F32, kind="ExternalInput").ap()
    biasr_d = nc.dram_tensor("biasr", [n_layers, 128, 2 * MT], F32, kind="ExternalInput").ap()
    whead_d = nc.dram_tensor("whead", [128, KC, V], MDT, kind="ExternalInput").ap()
    bhead_d = nc.dram_tensor("bheadc", [V, 1], F32, kind="ExternalInput").ap()
    lnc_d = nc.dram_tensor("lnc", [128, 2], F32, kind="ExternalInput").ap()
    cst_d = nc.dram_tensor("cst", [128, CST_W], MDT, kind="ExternalInput").ap()
    out_d = nc.dram_tensor("logitsT", [n_b, V, T], F32, kind="ExternalOutput").ap()

    from contextlib import ExitStack

    with tile.TileContext(nc) as tc, \
         nc.allow_low_precision(reason="bf16 matmul operand production"), \
         ExitStack() as ctx:
        ep = ctx.enter_context

        # ---------------- pools ----------------
        cpool = ep(tc.tile_pool(name="consts", bufs=1))
        xpool = ep(tc.tile_pool(name="x", bufs=1))
        wpool_qkv = ep(tc.tile_pool(name="wqkv", bufs=1))
        wpool_proj = ep(tc.tile_pool(name="wproj", bufs=1))
        wpool_1 = ep(tc.tile_pool(name="w1", bufs=1))
        wpool_2 = ep(tc.tile_pool(name="w2", bufs=1))
        wpool_b = ep(tc.tile_pool(name="wbias", bufs=1))
        hpool = ep(tc.tile_pool(name="h", bufs=3))
        xsqpool = ep(tc.tile_pool(name="xsq", bufs=2))
        hotpool = ep(tc.tile_pool(name="hot", bufs=2))
        qpool = ep(tc.tile_pool(name="q", bufs=2))
        kpool = ep(tc.tile_pool(name="k", bufs=2))
        vpool = ep(tc.tile_pool(name="v", bufs=1))
        upool = ep(tc.tile_pool(name="u", bufs=4))
        atmpool = ep(tc.tile_pool(name="atm", bufs=2))
        attpool = ep(tc.tile_pool(name="att", bufs=1))
        h1pool = ep(tc.tile_pool(name="h1", bufs=1))
        lgpool = ep(tc.tile_pool(name="lg", bufs=1))
        stpool = ep(tc.tile_pool(name="st", bufs=6))
        rdpool = ep(tc.tile_pool(name="rd", bufs=4))

        pbig = ep(tc.tile_pool(name="pbig", bufs=4, space="PSUM"))
        ppv = ep(tc.tile_pool(name="ppv", bufs=2, space="PSUM"))
        pstat = ep(tc.tile_pool(name="pstat", bufs=2, space="PSUM"))

        mm = _MM(nc)

        # ---------------- constants ----------------
        cst = cpool.tile([128, CST_W], MDT, name="cst_c")
        nc.sync.dma_start(out=cst[:, :], in_=cst_d[:, :])
        ones = cst[:, 0:512]
        ident = cst[:, 513:641]
        inv512 = cst[:, 641:769]
        mask = cpool.tile([128, 512], MDT, name="mask_c")
        nc.sync.dma_start(out=mask[:, :], in_=mask_d[:, :])
        whead = cpool.tile([128, KC, V], MDT, name="whead_c")
        nc.sync.dma_start(out=whead[:, :, :], in_=whead_d[:, :, :])
        bhead = cpool.tile([V, 1], F32, name="bhead_c")
        nc.sync.dma_start(out=bhead[:, :], in_=bhead_d[:, :])
        lnc = cpool.tile([128, 2], F32, name="lnc_c")
        nc.sync.dma_start(out=lnc[:, :], in_=lnc_d[:, :])

        x = xpool.tile([128, KC, ntok], MDT, name="x_resid")

        nbp = PW // T
        v_tiles = [vpool.tile([128, 2 * nbp, n_heads, HD1], MDT, name=f"v_pp{i}")
                   for i in range(2)]
        for vt in v_tiles:
            for tb in range(2 * nbp):
                nc.vector.tensor_copy(vt[:, tb, :, HD:HD1],
                                      ones[:, 0:2 * n_heads].rearrange(
                                          "p (h c) -> p h c", h=n_heads))

        # ---------------- LN (mean-free): sq -> mm+rstd -> finish ----------------
        def ln_sq(p):
            """x^2 on GpSimd (idle engine; keeps ACT free for exp/relu and
            never blocks the PE stat matmuls); emitted a phase early."""
            pc = slice(p * PW, p * PW + PW)
            xsq = xsqpool.tile([128, KC, PW], MDT, tag="xsq")
            nc.gpsimd.tensor_tensor(xsq[:, :, :], x[:, :, pc], x[:, :, pc],
                                    op=OP.mult)
            return (p, xsq)

        def ln_mmrstd(tok):
            """ones-matmul partition reduction -> Ln -> Exp -> rstd"""
            p, xsq = tok
            msqb = pstat.tile([128, PW], F32, tag="pstat")
            for c in range(KC):
                mm(msqb[:, :], inv512[:, :], xsq[:, c, :],
                   first=(c == 0), last=(c == KC - 1))
            lnv = stpool.tile([128, PW], F32, tag="st")
            nc.scalar.activation(lnv[:, :], msqb[:, :], AF.Ln, bias=lnc[:, 0:1])
            rstdb = stpool.tile([128, PW], MDT, tag="st")
            nc.scalar.activation(rstdb[:, :], lnv[:, :], AF.Exp,
                                 scale=-0.5, bias=lnc[:, 1:2])
            return (p, rstdb)

        def ln_stats(p):
            return ln_mmrstd(ln_sq(p))

        def ln_finish(tok):
            """apply -> h (pure DVE; no PE work)"""
            p, rstdb = tok
            pc = slice(p * PW, p * PW + PW)
            h = hpool.tile([128, KC, PW], MDT, tag="h")
            for c in range(KC):
                nc.vector.tensor_tensor(h[:, c, :], x[:, c, pc], rstdb[:, :], op=OP.mult)
            return h

        # ---------------- embedding (scratch tiles borrow other pools) ----------------
        pos = attpool.tile([128, KC, 512], MDT, tag="att")
        nc.sync.dma_start(out=pos[:, :, :], in_=pos_d[:, :, :])
        temb = stpool.tile([V, 384], MDT, tag="st")
        nc.sync.dma_start(out=temb[:, :], in_=temb_d[:, :])
        emb_sq = {}
        for ch in range(ntok // PW):
            cs = slice(ch * PW, ch * PW + PW)
            hot = hotpool.tile([V, PW], MDT, tag="hot")
            nc.sync.dma_start(out=hot[:, :], in_=hot_d[:, cs])
            for c in range(KC):
                ps = pbig.tile([128, 512], F32, tag="pbig")
                mm(ps[:, 0:PW], temb[0:V, c * 128:(c + 1) * 128], hot[0:V, :])
                nc.vector.tensor_tensor(x[:, c, cs], ps[:, 0:PW], pos[:, c, 0:PW], op=OP.add)
            if ch < 2:
                emb_sq[ch] = ln_sq(ch)   # overlap Square with later emb chunks

        def run(mids):
            out = []
            for f in mids:
                out.append(f())
            return out

        # ---------------- phase B: qkv + attention + proj for a pair ----------------
        def emit_B(p, h, wqkv, wproj, vbias, biasc, biasr, mid_a=(), mid_b=()):
            pc = slice(p * PW, p * PW + PW)
            nb_in_p = PW // T
            q_t = qpool.tile([128, MT, PW], MDT, tag="q")
            k_t = kpool.tile([128, MT, PW], MDT, tag="k")
            for mat, dst in ((0, q_t), (1, k_t)):
                for mt in range(MT):
                    ps = pbig.tile([128, 512], F32, tag="pbig")
                    for kc in range(KC):
                        mm(ps[:, 0:PW], wqkv[:, mat, kc, mt * 128:(mt + 1) * 128],
                           h[:, kc, :], first=(kc == 0), last=(kc == KC - 1))
                    nc.vector.tensor_scalar(
                        dst[:, mt, :], ps[:, 0:PW],
                        biasr[:, mat * MT + mt:mat * MT + mt + 1], None, op0=OP.add)
            v_t = v_tiles[p % 2]
            for tb in range(2 * nb_in_p):
                vps = pbig.tile([128, 512], F32, tag="pbig")
                for kc in range(KC):
                    mm(vps[:, 0:D], h[:, kc, tb * 128:(tb + 1) * 128],
                       wqkv[:, 2, kc, :], first=(kc == 0), last=(kc == KC - 1))
                nc.vector.tensor_tensor(
                    v_t[:, tb, :, 0:HD],
                    vps[:, 0:D].rearrange("p (h d) -> p h d", h=n_heads),
                    vbias[:, :].rearrange("p (h d) -> p h d", h=n_heads),
                    op=OP.add)
            mids_out = run(mid_a)
            atms = []
            for bi in range(nb_in_p):
                boff = bi * T
                if bi == 1:
                    mids_out += run(mid_b)
                us = [None] * n_heads
                pv0 = ppv.tile([128, n_heads, HD1], F32, tag="ppv")
                pv1 = ppv.tile([128, n_heads, HD1], F32, tag="ppv")

                def emit_S(hh):
                    hp = 64 * (hh % 2)
                    hc = hh // 2
                    sps = pbig.tile([128, 512], F32, tag="pbig")
                    mm(sps[:, 0:256], k_t[hp:hp + HD, hc, boff:boff + 128],
                       q_t[hp:hp + HD, hc, boff:boff + T])
                    mm(sps[:, 384:512], k_t[hp:hp + HD, hc, boff + 128:boff + 256],
                       q_t[hp:hp + HD, hc, boff + 128:boff + 256])
                    # cols 256:384 (keys 128-255 x queries 0-127) are fully
                    # masked and never read by PV: skip exp there entirely;
                    # cols 128:256 (keys 0-127 x queries 128-255) are fully
                    # valid: no mask multiply needed.
                    u_t = upool.tile([128, 512], MDT, tag="u")
                    nc.scalar.activation(u_t[:, 0:256], sps[:, 0:256], AF.Exp)
                    nc.scalar.activation(u_t[:, 384:512], sps[:, 384:512], AF.Exp)
                    nc.vector.tensor_tensor(u_t[:, 0:128], u_t[:, 0:128],
                                            mask[:, 0:128], op=OP.mult)
                    nc.vector.tensor_tensor(u_t[:, 384:512], u_t[:, 384:512],
                                            mask[:, 384:512], op=OP.mult)
                    us[hh] = u_t

                def emit_PV(hh):
                    u_t = us[hh]
                    mm(pv0[:, hh, :], u_t[:, 0:128], v_t[:, 2 * bi, hh, :])
                    mm(pv1[:, hh, :], u_t[:, 128:256], v_t[:, 2 * bi, hh, :],
                       first=True, last=False)
                    mm(pv1[:, hh, :], u_t[:, 384:512], v_t[:, 2 * bi + 1, hh, :],
                       first=False, last=True)

                # window the S/PV interleave so only ~3 U tiles are live
                emit_S(0); emit_S(1); emit_S(2)
                emit_PV(0); emit_S(3)
                emit_PV(1); emit_S(4)
                emit_PV(2); emit_S(5)
                emit_PV(3); emit_PV(4); emit_PV(5)
                atm = atmpool.tile([128, 2, n_heads * HD], MDT, tag="atm")
                for tb, pv in ((0, pv0), (1, pv1)):
                    rden = rdpool.tile([128, n_heads], F32, tag="rd")
                    nc.vector.reciprocal(rden[:, :], pv[:, :, HD])
                    nc.vector.tensor_tensor(
                        atm[:, tb, :].rearrange("p (h d) -> p h d", h=n_heads),
                        pv[:, :, 0:HD],
                        rden[:, :, None].broadcast_to([128, n_heads, HD]),
                        op=OP.mult)
                atms.append(atm)
            att_t = attpool.tile([128, KC, PW], MDT, tag="att")
            for c in range(KC):
                tps = pbig.tile([128, 512], MDT, tag="pbig")
                for bi in range(nb_in_p):
                    for tb in range(2):
                        col = (bi * 2 + tb) * 128
                        nc.tensor.transpose(
                            tps[:, col:col + 128],
                            atms[bi][:, tb, c * 128:(c + 1) * 128],
                            ident[:, :])
                nc.vector.tensor_copy(att_t[:, c, :], tps[:, 0:PW])
            for mt in range(MT):
                pp = pbig.tile([128, 512], F32, tag="pbig")
                for kc in range(KC):
                    mm(pp[:, 0:PW], wproj[:, kc, mt * 128:(mt + 1) * 128],
                       att_t[:, kc, :], first=(kc == 0), last=(kc == KC - 1))
                nc.vector.scalar_tensor_tensor(
                    x[:, mt, pc], pp[:, 0:PW], biasc[:, mt:mt + 1], x[:, mt, pc],
                    op0=OP.add, op1=OP.add)
            return mids_out

        # ---------------- phase D: FFN for a pair ----------------
        def emit_D(p, h2, w1, w2, biasc, mid=()):
            pc = slice(p * PW, p * PW + PW)
            h1_t = h1pool.tile([128, FT, PW], MDT, tag="h1")
            for mt in range(FT):
                fps = pbig.tile([128, 512], F32, tag="pbig")
                for kc in range(KC):
                    mm(fps[:, 0:PW], w1[:, kc, mt * 128:(mt + 1) * 128],
                       h2[:, kc, :], first=(kc == 0), last=(kc == KC - 1))
                nc.scalar.activation(h1_t[:, mt, :], fps[:, 0:PW], AF.Relu,
                                     bias=biasc[:, MT + mt:MT + mt + 1])
            mids_out = run(mid)
            for mt in range(MT):
                fp2 = pbig.tile([128, 512], F32, tag="pbig")
                for kc in range(FT):
                    mm(fp2[:, 0:PW], w2[:, kc, mt * 128:(mt + 1) * 128],
                       h1_t[:, kc, :], first=(kc == 0), last=(kc == FT - 1))
                nc.vector.scalar_tensor_tensor(
                    x[:, mt, pc], fp2[:, 0:PW],
                    biasc[:, MT + FT + mt:MT + FT + mt + 1],
                    x[:, mt, pc], op0=OP.add, op1=OP.add)
            return mids_out

        # ---------------- layers: software-pipelined emission ----------------
        carry = {}
        for l in range(n_layers):
            wqkv = wpool_qkv.tile([128, 3, KC, D], MDT, tag="wqkv")
            nc.sync.dma_start(out=wqkv[:, :, :, :], in_=wqkv_d[l])
            wproj = wpool_proj.tile([128, KC, D], MDT, tag="wproj")
            nc.sync.dma_start(out=wproj[:, :, :], in_=wproj_d[l])
            w1 = wpool_1.tile([128, KC, DFF], MDT, tag="w1")
            nc.sync.dma_start(out=w1[:, :, :], in_=w1_d[l])
            w2 = wpool_2.tile([128, FT, D], MDT, tag="w2")
            nc.sync.dma_start(out=w2[:, :, :], in_=w2_d[l])
            vbias = wpool_b.tile([128, D], F32, tag="vbias")
            nc.sync.dma_start(out=vbias[:, :], in_=vbias_d[l])
            biasc = wpool_b.tile([128, MT + FT + MT], F32, tag="biasc")
            nc.sync.dma_start(out=biasc[:, :], in_=biasc_d[l])
            biasr = wpool_b.tile([128, 2 * MT], F32, tag="biasr")
            nc.sync.dma_start(out=biasr[:, :], in_=biasr_d[l])

            Bf = lambda p, h, **kw: emit_B(p, h, wqkv, wproj, vbias, biasc, biasr, **kw)
            Dp = lambda p, h2, **kw: emit_D(p, h2, w1, w2, biasc, **kw)

            if npair == 4:
                if l == 0:
                    s0 = ln_mmrstd(emb_sq[0])
                    h0 = ln_finish(s0)
                    s1 = ln_mmrstd(emb_sq[1])
                else:
                    h0, s1 = carry["h0"], carry["s1"]
                # naming: sN = LN1 stats token pair N; scN = LN2 stats token
                (h1,) = Bf(0, h0, mid_a=[lambda: ln_finish(s1)])
                q0 = ln_sq(0)                   # LN2 p0 (x p0 updated by B0)
                h2_0, s2 = Bf(1, h1, mid_a=[lambda: ln_finish(ln_mmrstd(q0)),
                                            lambda: ln_mmrstd(ln_sq(2))])
                (g2,) = Dp(0, h2_0, mid=[lambda: ln_finish(s2)])
                q1 = ln_sq(1)                   # LN2 p1 (x p1 updated by B1)
                h2_1, s3 = Bf(2, g2, mid_a=[lambda: ln_finish(ln_mmrstd(q1)),
                                            lambda: ln_mmrstd(ln_sq(3))])
                (g3,) = Dp(1, h2_1, mid=[lambda: ln_finish(s3)])
                q2 = ln_sq(2)                   # LN2 p2 (x p2 updated by B2)
                (h2_2,) = Bf(3, g3, mid_a=[lambda: ln_finish(ln_mmrstd(q2))])
                last = (l == n_layers - 1)
                q3 = ln_sq(3)                   # LN2 p3 (x p3 updated by B3)
                if not last:
                    q0n = ln_sq(0)              # LN1-next p0 (x p0 upd by D0)
                    sc3, s0n = Dp(2, h2_2, mid=[lambda: ln_mmrstd(q3),
                                                lambda: ln_mmrstd(q0n)])
                    h2_3 = ln_finish(sc3)
                    q1n = ln_sq(1)              # LN1-next p1 (x p1 upd by D1)
                    h0n, s1n = Dp(3, h2_3, mid=[lambda: ln_finish(s0n),
                                                lambda: ln_mmrstd(q1n)])
                    carry = {"h0": h0n, "s1": s1n}
                else:
                    qf0 = ln_sq(0)              # final LN p0 (x p0 upd by D0)
                    sc3, sf0 = Dp(2, h2_2, mid=[lambda: ln_mmrstd(q3),
                                                lambda: ln_mmrstd(qf0)])
                    h2_3 = ln_finish(sc3)
                    qf1 = ln_sq(1)
                    hf0, sf1 = Dp(3, h2_3, mid=[lambda: ln_finish(sf0),
                                                lambda: ln_mmrstd(qf1)])
                    carry = {"hf0": hf0, "sf1": sf1}
            else:
                # simple order for small test configs
                hq = {}
                hq[0] = ln_finish(ln_stats(0))
                for p in range(1, npair):
                    hq[p] = ln_finish(ln_stats(p))
                    Bf(p - 1, hq.pop(p - 1))
                Bf(npair - 1, hq.pop(npair - 1))
                hq[0] = ln_finish(ln_stats(0))
                for p in range(1, npair):
                    hq[p] = ln_finish(ln_stats(p))
                    Dp(p - 1, hq.pop(p - 1))
                Dp(npair - 1, hq.pop(npair - 1))

        # ---------------- final LN + head ----------------
        def emit_head(p, hf):
            for bi in range(PW // T):
                b = p * (PW // T) + bi
                hps = ppv.tile([V, 256], F32, tag="ppv")
                for kc in range(KC):
                    mm(hps[:, :], whead[:, kc, :], hf[:, kc, bi * T:(bi + 1) * T],
                       first=(kc == 0), last=(kc == KC - 1))
                lg = lgpool.tile([V, T], F32, tag="lg")
                nc.vector.tensor_scalar(lg[:, :], hps[:, :], bhead[0:V, 0:1], None,
                                        op0=OP.add)
                nc.sync.dma_start(out=out_d[b], in_=lg[:, :])

        if npair == 4:
            hf, sf_next = carry["hf0"], carry["sf1"]
            for p in range(npair):
                if p + 2 < npair:
                    qn = ln_sq(p + 2)
                else:
                    qn = None
                emit_head(p, hf)
                if p + 1 < npair:
                    hf = ln_finish(sf_next)
                    if qn is not None:
                        sf_next = ln_mmrstd(qn)
        else:
            for p in range(npair):
                emit_head(p, ln_finish(ln_stats(p)))

    nc.compile()
    return nc


# ---------------------------------------------------------------------------
# host side
# ---------------------------------------------------------------------------

def prep_inputs(inputs, n_layers=L, n_b=BL, core=0):
    """Build the per-core input map (numpy) for `core`."""
    f32 = np.float32
    idx = np.asarray(inputs["idx"])
    tok_emb = np.asarray(inputs["tok_emb"], f32)
    pos_emb = np.asarray(inputs["pos_emb"], f32)
    Wq = np.asarray(inputs["Wq"], f32)
    Wk = np.asarray(inputs["Wk"], f32)
    Wv = np.asarray(inputs["Wv"], f32)
    Wproj = np.asarray(inputs["Wproj"], f32)
    bproj = np.asarray(inputs["bproj"], f32)
    W1 = np.asarray(inputs["W1"], f32)
    b1 = np.asarray(inputs["b1"], f32)
    W2 = np.asarray(inputs["W2"], f32)
    b2 = np.asarray(inputs["b2"], f32)
    ln1_g = np.asarray(inputs["ln1_g"], f32)
    ln1_b = np.asarray(inputs["ln1_b"], f32)
    ln2_g = np.asarray(inputs["ln2_g"], f32)
    ln2_b = np.asarray(inputs["ln2_b"], f32)
    lnf_g = np.asarray(inputs["lnf_g"], f32)
    lnf_b = np.asarray(inputs["lnf_b"], f32)
    Whead = np.asarray(inputs["Whead"], f32)
    bhead = np.asarray(inputs["bhead"], f32)

    ntok = n_b * T
    scale = f32(D) ** -0.5

    # mean-free residual: center embeddings + Wproj/W2 output columns
    tok_emb = tok_emb - tok_emb.mean(-1, keepdims=True)
    pos_emb = pos_emb - pos_emb.mean(-1, keepdims=True)
    Wproj = Wproj - Wproj.mean(-1, keepdims=True)
    W2 = W2 - W2.mean(-1, keepdims=True)
    bproj = bproj - bproj.mean(-1, keepdims=True)
    b2 = b2 - b2.mean(-1, keepdims=True)

    idx_c = idx[core * n_b:(core + 1) * n_b].reshape(-1)         # [ntok]
    hot = (idx_c[None, :] == np.arange(V)[:, None]).astype(NPBF)  # [V, ntok]

    posT = pos_emb.T.astype(f32)                                 # [D, T]
    posT2 = np.concatenate([posT, posT], axis=1)                 # [D, 512]
    pos_in = posT2.reshape(KC, 128, 512).transpose(1, 0, 2).astype(NPBF)

    lane = np.arange(128)
    t = np.arange(T)
    m0 = (lane[:, None] <= t[None, :]).astype(NPBF)
    m1 = ((lane[:, None] + 128) <= t[None, :]).astype(NPBF)
    mask = np.concatenate([m0, m1], axis=1)                      # [128, 512]

    def pack_w(w):  # [D_in, N] -> [128, KC_in, N]
        kin = w.shape[0] // 128
        return w.reshape(kin, 128, -1).transpose(1, 0, 2).copy()

    wqkv = np.zeros((n_layers, 128, 3, KC, D), NPBF)
    wproj = np.zeros((n_layers, 128, KC, D), NPBF)
    w1 = np.zeros((n_layers, 128, KC, DFF), NPBF)
    w2 = np.zeros((n_layers, 128, FT, D), NPBF)
    vbias = np.zeros((n_layers, 128, D), f32)
    biasc = np.zeros((n_layers, 128, MT + FT + MT), f32)
    biasr = np.zeros((n_layers, 128, 2 * MT), f32)

    for l in range(n_layers):
        # Wq[l] is [H, D, HD]; feature f = h*HD+hd -> transpose to [D, H, HD]
        wq2 = Wq[l].transpose(1, 0, 2).reshape(D, D) * scale
        wk2 = Wk[l].transpose(1, 0, 2).reshape(D, D)
        wv2 = Wv[l].transpose(1, 0, 2).reshape(D, D)
        wqkv[l, :, 0] = pack_w(wq2 * ln1_g[l][:, None])
        wqkv[l, :, 1] = pack_w(wk2 * ln1_g[l][:, None])
        wqkv[l, :, 2] = pack_w(wv2 * ln1_g[l][:, None])
        biasr[l, :, 0:MT] = (ln1_b[l] @ wq2).reshape(MT, 128).T
        biasr[l, :, MT:2 * MT] = (ln1_b[l] @ wk2).reshape(MT, 128).T
        vbias[l] = np.broadcast_to(ln1_b[l] @ wv2, (128, D))
        wproj[l] = pack_w(Wproj[l])
        w1[l] = pack_w(W1[l] * ln2_g[l][:, None])
        w2[l] = pack_w(W2[l])
        biasc[l, :, 0:MT] = bproj[l].reshape(MT, 128).T
        biasc[l, :, MT:MT + FT] = (b1[l] + ln2_b[l] @ W1[l]).reshape(FT, 128).T
        biasc[l, :, MT + FT:] = b2[l].reshape(MT, 128).T

    whead_eff = Whead * lnf_g[:, None]
    bhead_eff = (bhead + lnf_b @ Whead).astype(f32)

    cst = np.ones((128, CST_W), NPBF)
    cst[:, 513:641] = np.eye(128, dtype=NPBF)
    cst[:, 641:769] = NPBF(1.0 / 512.0)

    lnc = np.zeros((128, 2), f32)
    lnc[:, 0] = EPS_EFF
    lnc[:, 1] = RSTD_BIAS

    return {
        "cst": cst,
        "lnc": lnc,
        "hotT": hot,
        "temb": tok_emb.astype(NPBF),
        "posT": pos_in,
        "mask": mask,
        "wqkv": wqkv,
        "wproj": wproj,
        "w1": w1,
        "w2": w2,
        "vbias": vbias,
        "biasc": biasc,
        "biasr": biasr,
        "whead": pack_w(whead_eff).astype(NPBF),
        "bheadc": bhead_eff[:, None].copy(),
    }


_CACHE = {}


def get_program():
    if "nc" not in _CACHE:
        _CACHE["nc"] = build_program()
    return _CACHE["nc"]


def run_on_hw(inputs, trace=False):
    nc = get_program()
    in_maps = [prep_inputs(inputs, core=c) for c in range(NCORES)]
    res = run_bass_kernel_spmd(nc, in_maps, list(range(NCORES)), trace=trace)
    outs = []
    for c in range(NCORES):
        lt = res.results[c]["logitsT"]          # [BL, V, T]
        outs.append(lt.transpose(0, 2, 1))      # [BL, T, V]
    full = np.concatenate(outs, axis=0)         # [B, T, V]
    return full, res


def kernel(**inputs):
    out, _ = run_on_hw(inputs, trace=False)
    return out


# revision 20
# speedup vs baseline: 1.0064x; 1.0062x over previous
"""Trainium2 Bass kernel: GPT-style transformer forward pass.

Strategy: data-parallel over batch across 8 NeuronCores (B=64 -> 8 per core),
weights replicated.  All matmuls/activations in bf16 (PE bf16 = 1 cyc/row at
any moving-dim size, DVE 2x on 16-bit, half DMA/SBUF), PSUM accumulation f32.

Mean-free residual trick: tok/pos embeddings and the output columns of
Wproj/W2 (+ bproj/b2) are centered over the feature dim host-side, so the
residual stream x stays exactly zero-mean and LayerNorm needs NO mean
subtraction (logits are invariant: every consumer of x is a LayerNorm).
LN reduces to rstd = (E[x^2]+eps)^-1/2 via one ones-matmul of x^2 and
h = x * rstd.  Q/K biases are applied on the PSUM->SBUF copy (per-partition
bias columns) instead of via rank-1 matmuls.

Attention: transposed scores S_T[s,t] per (b,h), multiplicative causal mask
after exp, token-major PV in bf16 (moving dim 66 at full bf16 rate) with an
appended ones-column in V so softmax denominators land as a per-partition
column, and bf16 PE transposes carry att back to feature-major.
"""

import os
import sys

for _p in ("/opt/trn_rl_repo",):
    if _p not in sys.path and os.path.isdir(_p):
        sys.path.insert(0, _p)

import numpy as np
import ml_dtypes

import concourse.bass as bass
import concourse.mybir as mybir
import concourse.tile as tile
from concourse import bacc
from concourse.bass_utils import run_bass_kernel_spmd

F32 = mybir.dt.float32
BF16 = mybir.dt.bfloat16
AF = mybir.ActivationFunctionType
OP = mybir.AluOpType

NPBF = ml_dtypes.bfloat16

V, D, H, HD, L, T, B = 65, 384, 6, 64, 6, 256, 64
NCORES = 8
BL = B // NCORES          # batch elements per core
NTOK = BL * T             # tokens per core
DFF = 4 * D               # 1536
EPS = 1e-3
KC = D // 128             # 3 contraction chunks of 128
MT = D // 128             # 3 output feature tiles
FT = DFF // 128           # 12 ffn tiles
HD1 = HD + 2              # V columns incl ones col (64) + pad (65)
MDT = BF16

# stat matmul uses 1/512 (exact in bf16); corrections folded into Ln/Exp:
#   msq = sum(x^2)/512 = var * D/512
#   rstd = exp(-0.5*ln(msq + EPS*D/512) + 0.5*ln(D/512))
EPS_EFF = EPS * D / 512.0
RSTD_BIAS = 0.5 * float(np.log(D / 512.0))

# cst layout: cols 0:512 ones, col 512 unused, cols 513:641 identity 128x128,
# cols 641:769 = 1/512 block (stats matmul lhsT)
CST_W = 513 + 128 + 128


def _patch_act_tables():
    """Steer the activation-table picker to natural_log_exp_and_others for
    Exp and Ln, so this kernel's ACT stream never switches table sets."""
    if getattr(bacc, "_act_tables_patched", False):
        return
    real = bacc.get_activation_tables

    def patched(arch):
        t = real(arch)
        exp = mybir.ActivationFunctionType.Exp
        ln = mybir.ActivationFunctionType.Ln
        out = {}
        for name, fns in t.items():
            if name != "natural_log_exp_and_others":
                fns = fns - {exp, ln}
            out[name] = fns
        return out

    bacc.get_activation_tables = patched
    bacc._act_tables_patched = True


class _MM:
    """matmul emitter with explicit accumulation-chain boundaries."""

    def __init__(self, nc):
        self.nc = nc

    def __call__(self, out, lhsT, rhs, first=True, last=True, tile_position=None):
        self.nc.tensor.matmul(
            out, lhsT, rhs, start=first, stop=last, tile_position=tile_position,
        )


def build_program(n_layers=L, n_b=BL, n_heads=H):
    _patch_act_tables()
    assert n_b % 2 == 0 or n_b == 1
    ntok = n_b * T
    npair = max(1, n_b // 2)
    PW = 512 if n_b > 1 else 256      # tokens per pair-chunk
    nc = bacc.Bacc("TRN2", target_bir_lowering=False, debug=False)

    # ---------------- DRAM I/O ----------------
    hot_d = nc.dram_tensor("hotT", [V, ntok], MDT, kind="ExternalInput").ap()
    temb_d = nc.dram_tensor("temb", [V, D], MDT, kind="ExternalInput").ap()
    pos_d = nc.dram_tensor("posT", [128, KC, 512], MDT, kind="ExternalInput").ap()
    mask_d = nc.dram_tensor("mask", [128, 384], MDT, kind="ExternalInput").ap()
    wqkv_d = nc.dram_tensor("wqkv", [n_layers, 128, 3, KC, D], MDT, kind="ExternalInput").ap()
    wproj_d = nc.dram_tensor("wproj", [n_layers, 128, KC, D], MDT, kind="ExternalInput").ap()
    w1_d = nc.dram_tensor("w1", [n_layers, 128, KC, DFF], MDT, kind="ExternalInput").ap()
    w2_d = nc.dram_tensor("w2", [n_layers, 128, FT, D], MDT, kind="ExternalInput").ap()
    vbias_d = nc.dram_tensor("vbias", [n_layers, 128, D], F32, kind="ExternalInput").ap()
    biasc_d = nc.dram_tensor("biasc", [n_layers, 128, MT + FT + MT], F32, kind="ExternalInput").ap()
    biasr_d = nc.dram_tensor("biasr", [n_layers, 128, 2 * MT], F32, kind="ExternalInput").ap()
    whead_d = nc.dram_tensor("whead", [128, KC, V], MDT, kind="ExternalInput").ap()
    bhead_d = nc.dram_tensor("bheadc", [V, 1], F32, kind="ExternalInput").ap()
    lnc_d = nc.dram_tensor("lnc", [128, 2], F32, kind="ExternalInput").ap()
    cst_d = nc.dram_tensor("cst", [128, CST_W], MDT, kind="ExternalInput").ap()
    out_d = nc.dram_tensor("logitsT", [n_b, V, T], F32, kind="ExternalOutput").ap()

    from contextlib import ExitStack

    with tile.TileContext(nc) as tc, \
         nc.allow_low_precision(reason="bf16 matmul operand production"), \
         ExitStack() as ctx:
        ep = ctx.enter_context

        # ---------------- pools ----------------
        cpool = ep(tc.tile_pool(name="consts", bufs=1))
        xpool = ep(tc.tile_pool(name="x", bufs=1))
        wpool_qkv = ep(tc.tile_pool(name="wqkv", bufs=1))
        wpool_proj = ep(tc.tile_pool(name="wproj", bufs=1))
        wpool_1 = ep(tc.tile_pool(name="w1", bufs=1))
        wpool_2 = ep(tc.tile_pool(name="w2", bufs=1))
        wpool_b = ep(tc.tile_pool(name="wbias", bufs=1))
        hpool = ep(tc.tile_pool(name="h", bufs=3))
        xsqpool = ep(tc.tile_pool(name="xsq", bufs=2))
        hotpool = ep(tc.tile_pool(name="hot", bufs=2))
        qpool = ep(tc.tile_pool(name="q", bufs=2))
        kpool = ep(tc.tile_pool(name="k", bufs=2))
        vpool = ep(tc.tile_pool(name="v", bufs=1))
        upool = ep(tc.tile_pool(name="u", bufs=6))
        atmpool = ep(tc.tile_pool(name="atm", bufs=2))
        attpool = ep(tc.tile_pool(name="att", bufs=1))
        h1pool = ep(tc.tile_pool(name="h1", bufs=1))
        lgpool = ep(tc.tile_pool(name="lg", bufs=1))
        stpool = ep(tc.tile_pool(name="st", bufs=6))
        rdpool = ep(tc.tile_pool(name="rd", bufs=4))

        pbig = ep(tc.tile_pool(name="pbig", bufs=3, space="PSUM"))
        spool = ep(tc.tile_pool(name="sps", bufs=2, space="PSUM"))
        ppv = ep(tc.tile_pool(name="ppv", bufs=2, space="PSUM"))
        pstat = ep(tc.tile_pool(name="pstat", bufs=1, space="PSUM"))

        mm = _MM(nc)

        # ---------------- constants ----------------
        cst = cpool.tile([128, CST_W], MDT, name="cst_c")
        nc.sync.dma_start(out=cst[:, :], in_=cst_d[:, :])
        ones = cst[:, 0:512]
        ident = cst[:, 513:641]
        inv512 = cst[:, 641:769]
        mask = cpool.tile([128, 384], MDT, name="mask_c")
        nc.sync.dma_start(out=mask[:, :], in_=mask_d[:, :])
        whead = cpool.tile([128, KC, V], MDT, name="whead_c")
        nc.sync.dma_start(out=whead[:, :, :], in_=whead_d[:, :, :])
        bhead = cpool.tile([V, 1], F32, name="bhead_c")
        nc.sync.dma_start(out=bhead[:, :], in_=bhead_d[:, :])
        lnc = cpool.tile([128, 2], F32, name="lnc_c")
        nc.sync.dma_start(out=lnc[:, :], in_=lnc_d[:, :])

        x = xpool.tile([128, KC, ntok], MDT, name="x_resid")

        nbp = PW // T
        v_tiles = [vpool.tile([128, 2 * nbp, n_heads, HD1], MDT, name=f"v_pp{i}")
                   for i in range(2)]
        for vt in v_tiles:
            for tb in range(2 * nbp):
                nc.vector.tensor_copy(vt[:, tb, :, HD:HD1],
                                      ones[:, 0:2 * n_heads].rearrange(
                                          "p (h c) -> p h c", h=n_heads))

        # ---------------- LN (mean-free): sq -> mm+rstd -> finish ----------------
        def ln_sq(p):
            """x^2 on GpSimd (idle engine; keeps ACT free for exp/relu and
            never blocks the PE stat matmuls); emitted a phase early."""
            pc = slice(p * PW, p * PW + PW)
            xsq = xsqpool.tile([128, KC, PW], MDT, tag="xsq")
            nc.gpsimd.tensor_tensor(xsq[:, :, :], x[:, :, pc], x[:, :, pc],
                                    op=OP.mult)
            return (p, xsq)

        def ln_mmrstd(tok):
            """ones-matmul partition reduction -> Ln -> Exp -> rstd"""
            p, xsq = tok
            msqb = pstat.tile([128, PW], F32, tag="pstat")
            for c in range(KC):
                mm(msqb[:, :], inv512[:, :], xsq[:, c, :],
                   first=(c == 0), last=(c == KC - 1))
            lnv = stpool.tile([128, PW], F32, tag="st")
            nc.scalar.activation(lnv[:, :], msqb[:, :], AF.Ln, bias=lnc[:, 0:1])
            rstdb = stpool.tile([128, PW], MDT, tag="st")
            nc.scalar.activation(rstdb[:, :], lnv[:, :], AF.Exp,
                                 scale=-0.5, bias=lnc[:, 1:2])
            return (p, rstdb)

        def ln_stats(p):
            return ln_mmrstd(ln_sq(p))

        def ln_finish(tok):
            """apply -> h (pure DVE; no PE work)"""
            p, rstdb = tok
            pc = slice(p * PW, p * PW + PW)
            h = hpool.tile([128, KC, PW], MDT, tag="h")
            for c in range(KC):
                nc.vector.tensor_tensor(h[:, c, :], x[:, c, pc], rstdb[:, :], op=OP.mult)
            return h

        # ---------------- embedding (scratch tiles borrow other pools) ----------------
        pos = attpool.tile([128, KC, 512], MDT, tag="att")
        nc.sync.dma_start(out=pos[:, :, :], in_=pos_d[:, :, :])
        temb = stpool.tile([V, 384], MDT, tag="st")
        nc.sync.dma_start(out=temb[:, :], in_=temb_d[:, :])
        emb_sq = {}
        for ch in range(ntok // PW):
            cs = slice(ch * PW, ch * PW + PW)
            hot = hotpool.tile([V, PW], MDT, tag="hot")
            nc.sync.dma_start(out=hot[:, :], in_=hot_d[:, cs])
            for c in range(KC):
                ps = pbig.tile([128, 512], F32, tag="pbig")
                mm(ps[:, 0:PW], temb[0:V, c * 128:(c + 1) * 128], hot[0:V, :])
                nc.vector.tensor_tensor(x[:, c, cs], ps[:, 0:PW], pos[:, c, 0:PW], op=OP.add)
            if ch < 2:
                emb_sq[ch] = ln_sq(ch)   # overlap Square with later emb chunks

        def run(mids):
            out = []
            for f in mids:
                out.append(f())
            return out

        # ---------------- phase B: qkv + attention + proj for a pair ----------------
        def emit_B(p, h, wqkv, wproj, vbias, biasc, biasr, mid_a=(), mid_b=()):
            pc = slice(p * PW, p * PW + PW)
            nb_in_p = PW // T
            q_t = qpool.tile([128, MT, PW], MDT, tag="q")
            k_t = kpool.tile([128, MT, PW], MDT, tag="k")
            for mat, dst in ((0, q_t), (1, k_t)):
                for mt in range(MT):
                    ps = pbig.tile([128, 512], F32, tag="pbig")
                    for kc in range(KC):
                        mm(ps[:, 0:PW], wqkv[:, mat, kc, mt * 128:(mt + 1) * 128],
                           h[:, kc, :], first=(kc == 0), last=(kc == KC - 1))
                    nc.scalar.activation(
                        dst[:, mt, :], ps[:, 0:PW], AF.Identity,
                        bias=biasr[:, mat * MT + mt:mat * MT + mt + 1])
            v_t = v_tiles[p % 2]
            for tb in range(2 * nb_in_p):
                vps = pbig.tile([128, 512], F32, tag="pbig")
                for kc in range(KC):
                    mm(vps[:, 0:D], h[:, kc, tb * 128:(tb + 1) * 128],
                       wqkv[:, 2, kc, :], first=(kc == 0), last=(kc == KC - 1))
                nc.vector.tensor_tensor(
                    v_t[:, tb, :, 0:HD],
                    vps[:, 0:D].rearrange("p (h d) -> p h d", h=n_heads),
                    vbias[:, :].rearrange("p (h d) -> p h d", h=n_heads),
                    op=OP.add)
            mids_out = run(mid_a)
            atms = []
            for bi in range(nb_in_p):
                boff = bi * T
                if bi == 1:
                    mids_out += run(mid_b)
                us = [None] * n_heads
                pv0 = ppv.tile([128, n_heads, HD1], F32, tag="ppv")
                pv1 = ppv.tile([128, n_heads, HD1], F32, tag="ppv")

                def emit_S(hh):
                    hp = 64 * (hh % 2)
                    hc = hh // 2
                    # packed score layout [128, 384]:
                    #   cols 0:256   keys 0-127  x queries 0-255
                    #   cols 256:384 keys 128-255 x queries 128-255
                    # (keys 128-255 x queries 0-127 is fully masked: skipped)
                    sps = spool.tile([128, 384], F32, tag="sps")
                    mm(sps[:, 0:256], k_t[hp:hp + HD, hc, boff:boff + 128],
                       q_t[hp:hp + HD, hc, boff:boff + T])
                    mm(sps[:, 256:384], k_t[hp:hp + HD, hc, boff + 128:boff + 256],
                       q_t[hp:hp + HD, hc, boff + 128:boff + 256])
                    u_t = upool.tile([128, 384], MDT, tag="u")
                    nc.scalar.activation(u_t[:, :], sps[:, :], AF.Exp)
                    # cols 128:256 (keys 0-127 x queries 128-255) are fully
                    # valid: no mask multiply needed there.
                    nc.vector.tensor_tensor(u_t[:, 0:128], u_t[:, 0:128],
                                            mask[:, 0:128], op=OP.mult)
                    nc.vector.tensor_tensor(u_t[:, 256:384], u_t[:, 256:384],
                                            mask[:, 256:384], op=OP.mult)
                    us[hh] = u_t

                def emit_PV(hh):
                    u_t = us[hh]
                    mm(pv0[:, hh, :], u_t[:, 0:128], v_t[:, 2 * bi, hh, :])
                    mm(pv1[:, hh, :], u_t[:, 128:256], v_t[:, 2 * bi, hh, :],
                       first=True, last=False)
                    mm(pv1[:, hh, :], u_t[:, 256:384], v_t[:, 2 * bi + 1, hh, :],
                       first=False, last=True)

                # window the S/PV interleave so only ~3 U tiles are live
                emit_S(0); emit_S(1); emit_S(2)
                emit_PV(0); emit_S(3)
                emit_PV(1); emit_S(4)
                emit_PV(2); emit_S(5)
                emit_PV(3); emit_PV(4); emit_PV(5)
                atm = atmpool.tile([128, 2, n_heads * HD], MDT, tag="atm")
                for tb, pv in ((0, pv0), (1, pv1)):
                    rden = rdpool.tile([128, n_heads], F32, tag="rd")
                    nc.vector.reciprocal(rden[:, :], pv[:, :, HD])
                    nc.vector.tensor_tensor(
                        atm[:, tb, :].rearrange("p (h d) -> p h d", h=n_heads),
                        pv[:, :, 0:HD],
                        rden[:, :, None].broadcast_to([128, n_heads, HD]),
                        op=OP.mult)
                atms.append(atm)
            att_t = attpool.tile([128, KC, PW], MDT, tag="att")
            for c in range(KC):
                tps = pbig.tile([128, 512], MDT, tag="pbig")
                for bi in range(nb_in_p):
                    for tb in range(2):
                        col = (bi * 2 + tb) * 128
                        nc.tensor.transpose(
                            tps[:, col:col + 128],
                            atms[bi][:, tb, c * 128:(c + 1) * 128],
                            ident[:, :])
                nc.vector.tensor_copy(att_t[:, c, :], tps[:, 0:PW])
            for mt in range(MT):
                pp = pbig.tile([128, 512], F32, tag="pbig")
                for kc in range(KC):
                    mm(pp[:, 0:PW], wproj[:, kc, mt * 128:(mt + 1) * 128],
                       att_t[:, kc, :], first=(kc == 0), last=(kc == KC - 1))
                nc.vector.scalar_tensor_tensor(
                    x[:, mt, pc], pp[:, 0:PW], biasc[:, mt:mt + 1], x[:, mt, pc],
                    op0=OP.add, op1=OP.add)
            return mids_out

        # ---------------- phase D: FFN for a pair ----------------
        def emit_D(p, h2, w1, w2, biasc, mid=()):
            pc = slice(p * PW, p * PW + PW)
            h1_t = h1pool.tile([128, FT, PW], MDT, tag="h1")
            for mt in range(FT):
                fps = pbig.tile([128, 512], F32, tag="pbig")
                for kc in range(KC):
                    mm(fps[:, 0:PW], w1[:, kc, mt * 128:(mt + 1) * 128],
                       h2[:, kc, :], first=(kc == 0), last=(kc == KC - 1))
                nc.scalar.activation(h1_t[:, mt, :], fps[:, 0:PW], AF.Relu,
                                     bias=biasc[:, MT + mt:MT + mt + 1])
            mids_out = run(mid)
            for mt in range(MT):
                fp2 = pbig.tile([128, 512], F32, tag="pbig")
                for kc in range(FT):
                    mm(fp2[:, 0:PW], w2[:, kc, mt * 128:(mt + 1) * 128],
                       h1_t[:, kc, :], first=(kc == 0), last=(kc == FT - 1))
                nc.vector.scalar_tensor_tensor(
                    x[:, mt, pc], fp2[:, 0:PW],
                    biasc[:, MT + FT + mt:MT + FT + mt + 1],
                    x[:, mt, pc], op0=OP.add, op1=OP.add)
            return mids_out

        # ---------------- layers: software-pipelined emission ----------------
        carry = {}
        for l in range(n_layers):
            wqkv = wpool_qkv.tile([128, 3, KC, D], MDT, tag="wqkv")
            nc.sync.dma_start(out=wqkv[:, :, :, :], in_=wqkv_d[l])
            wproj = wpool_proj.tile([128, KC, D], MDT, tag="wproj")
            nc.sync.dma_start(out=wproj[:, :, :], in_=wproj_d[l])
            w1 = wpool_1.tile([128, KC, DFF], MDT, tag="w1")
            nc.sync.dma_start(out=w1[:, :, :], in_=w1_d[l])
            w2 = wpool_2.tile([128, FT, D], MDT, tag="w2")
            nc.sync.dma_start(out=w2[:, :, :], in_=w2_d[l])
            vbias = wpool_b.tile([128, D], F32, tag="vbias")
            nc.sync.dma_start(out=vbias[:, :], in_=vbias_d[l])
            biasc = wpool_b.tile([128, MT + FT + MT], F32, tag="biasc")
            nc.sync.dma_start(out=biasc[:, :], in_=biasc_d[l])
            biasr = wpool_b.tile([128, 2 * MT], F32, tag="biasr")
            nc.sync.dma_start(out=biasr[:, :], in_=biasr_d[l])

            Bf = lambda p, h, **kw: emit_B(p, h, wqkv, wproj, vbias, biasc, biasr, **kw)
            Dp = lambda p, h2, **kw: emit_D(p, h2, w1, w2, biasc, **kw)

            if npair == 4:
                if l == 0:
                    s0 = ln_mmrstd(emb_sq[0])
                    h0 = ln_finish(s0)
                    s1 = ln_mmrstd(emb_sq[1])
                else:
                    h0, s1 = carry["h0"], carry["s1"]
                # naming: sN = LN1 stats token pair N; scN = LN2 stats token
                (h1,) = Bf(0, h0, mid_a=[lambda: ln_finish(s1)])
                q0 = ln_sq(0)                   # LN2 p0 (x p0 updated by B0)
                h2_0, s2 = Bf(1, h1, mid_a=[lambda: ln_finish(ln_mmrstd(q0)),
                                            lambda: ln_mmrstd(ln_sq(2))])
                (g2,) = Dp(0, h2_0, mid=[lambda: ln_finish(s2)])
                q1 = ln_sq(1)                   # LN2 p1 (x p1 updated by B1)
                h2_1, s3 = Bf(2, g2, mid_a=[lambda: ln_finish(ln_mmrstd(q1)),
                                            lambda: ln_mmrstd(ln_sq(3))])
                (g3,) = Dp(1, h2_1, mid=[lambda: ln_finish(s3)])
                q2 = ln_sq(2)                   # LN2 p2 (x p2 updated by B2)
                (h2_2,) = Bf(3, g3, mid_a=[lambda: ln_finish(ln_mmrstd(q2))])
                last = (l == n_layers - 1)
                q3 = ln_sq(3)                   # LN2 p3 (x p3 updated by B3)
                if not last:
                    q0n = ln_sq(0)              # LN1-next p0 (x p0 upd by D0)
                    sc3, s0n = Dp(2, h2_2, mid=[lambda: ln_mmrstd(q3),
                                                lambda: ln_mmrstd(q0n)])
                    h2_3 = ln_finish(sc3)
                    q1n = ln_sq(1)              # LN1-next p1 (x p1 upd by D1)
                    h0n, s1n = Dp(3, h2_3, mid=[lambda: ln_finish(s0n),
                                                lambda: ln_mmrstd(q1n)])
                    carry = {"h0": h0n, "s1": s1n}
                else:
                    qf0 = ln_sq(0)              # final LN p0 (x p0 upd by D0)
                    sc3, sf0 = Dp(2, h2_2, mid=[lambda: ln_mmrstd(q3),
                                                lambda: ln_mmrstd(qf0)])
                    h2_3 = ln_finish(sc3)
                    qf1 = ln_sq(1)
                    hf0, sf1 = Dp(3, h2_3, mid=[lambda: ln_finish(sf0),
                                                lambda: ln_mmrstd(qf1)])
                    carry = {"hf0": hf0, "sf1": sf1}
            else:
                # simple order for small test configs
                hq = {}
                hq[0] = ln_finish(ln_stats(0))
                for p in range(1, npair):
                    hq[p] = ln_finish(ln_stats(p))
                    Bf(p - 1, hq.pop(p - 1))
                Bf(npair - 1, hq.pop(npair - 1))
                hq[0] = ln_finish(ln_stats(0))
                for p in range(1, npair):
                    hq[p] = ln_finish(ln_stats(p))
                    Dp(p - 1, hq.pop(p - 1))
                Dp(npair - 1, hq.pop(npair - 1))

        # ---------------- final LN + head ----------------
        def emit_head(p, hf):
            for bi in range(PW // T):
                b = p * (PW // T) + bi
                hps = ppv.tile([V, 256], F32, tag="ppv")
                for kc in range(KC):
                    mm(hps[:, :], whead[:, kc, :], hf[:, kc, bi * T:(bi + 1) * T],
                       first=(kc == 0), last=(kc == KC - 1))
                lg = lgpool.tile([V, T], F32, tag="lg")
                nc.vector.tensor_scalar(lg[:, :], hps[:, :], bhead[0:V, 0:1], None,
                                        op0=OP.add)
                nc.sync.dma_start(out=out_d[b], in_=lg[:, :])

        if npair == 4:
            hf, sf_next = carry["hf0"], carry["sf1"]
            for p in range(npair):
                if p + 2 < npair:
                    qn = ln_sq(p + 2)
                else:
                    qn = None
                emit_head(p, hf)
                if p + 1 < npair:
                    hf = ln_finish(sf_next)
                    if qn is not None:
                        sf_next = ln_mmrstd(qn)
        else:
            for p in range(npair):
                emit_head(p, ln_finish(ln_stats(p)))

    nc.compile()
    return nc


# ---------------------------------------------------------------------------
# host side
# ---------------------------------------------------------------------------

def prep_inputs(inputs, n_layers=L, n_b=BL, core=0):
    """Build the per-core input map (numpy) for `core`."""
    f32 = np.float32
    idx = np.asarray(inputs["idx"])
    tok_emb = np.asarray(inputs["tok_emb"], f32)
    pos_emb = np.asarray(inputs["pos_emb"], f32)
    Wq = np.asarray(inputs["Wq"], f32)
    Wk = np.asarray(inputs["Wk"], f32)
    Wv = np.asarray(inputs["Wv"], f32)
    Wproj = np.asarray(inputs["Wproj"], f32)
    bproj = np.asarray(inputs["bproj"], f32)
    W1 = np.asarray(inputs["W1"], f32)
    b1 = np.asarray(inputs["b1"], f32)
    W2 = np.asarray(inputs["W2"], f32)
    b2 = np.asarray(inputs["b2"], f32)
    ln1_g = np.asarray(inputs["ln1_g"], f32)
    ln1_b = np.asarray(inputs["ln1_b"], f32)
    ln2_g = np.asarray(inputs["ln2_g"], f32)
    ln2_b = np.asarray(inputs["ln2_b"], f32)
    lnf_g = np.asarray(inputs["lnf_g"], f32)
    lnf_b = np.asarray(inputs["lnf_b"], f32)
    Whead = np.asarray(inputs["Whead"], f32)
    bhead = np.asarray(inputs["bhead"], f32)

    ntok = n_b * T
    scale = f32(D) ** -0.5

    # mean-free residual: center embeddings + Wproj/W2 output columns
    tok_emb = tok_emb - tok_emb.mean(-1, keepdims=True)
    pos_emb = pos_emb - pos_emb.mean(-1, keepdims=True)
    Wproj = Wproj - Wproj.mean(-1, keepdims=True)
    W2 = W2 - W2.mean(-1, keepdims=True)
    bproj = bproj - bproj.mean(-1, keepdims=True)
    b2 = b2 - b2.mean(-1, keepdims=True)

    idx_c = idx[core * n_b:(core + 1) * n_b].reshape(-1)         # [ntok]
    hot = (idx_c[None, :] == np.arange(V)[:, None]).astype(NPBF)  # [V, ntok]

    posT = pos_emb.T.astype(f32)                                 # [D, T]
    posT2 = np.concatenate([posT, posT], axis=1)                 # [D, 512]
    pos_in = posT2.reshape(KC, 128, 512).transpose(1, 0, 2).astype(NPBF)

    lane = np.arange(128)
    t = np.arange(T)
    m0 = (lane[:, None] <= t[None, :]).astype(NPBF)              # [128, 256]
    m1 = ((lane[:, None] + 128) <= t[None, 128:]).astype(NPBF)   # [128, 128]
    mask = np.concatenate([m0[:, 0:256], m1], axis=1)            # [128, 384]

    def pack_w(w):  # [D_in, N] -> [128, KC_in, N]
        kin = w.shape[0] // 128
        return w.reshape(kin, 128, -1).transpose(1, 0, 2).copy()

    wqkv = np.zeros((n_layers, 128, 3, KC, D), NPBF)
    wproj = np.zeros((n_layers, 128, KC, D), NPBF)
    w1 = np.zeros((n_layers, 128, KC, DFF), NPBF)
    w2 = np.zeros((n_layers, 128, FT, D), NPBF)
    vbias = np.zeros((n_layers, 128, D), f32)
    biasc = np.zeros((n_layers, 128, MT + FT + MT), f32)
    biasr = np.zeros((n_layers, 128, 2 * MT), f32)

    for l in range(n_layers):
        # Wq[l] is [H, D, HD]; feature f = h*HD+hd -> transpose to [D, H, HD]
        wq2 = Wq[l].transpose(1, 0, 2).reshape(D, D) * scale
        wk2 = Wk[l].transpose(1, 0, 2).reshape(D, D)
        wv2 = Wv[l].transpose(1, 0, 2).reshape(D, D)
        wqkv[l, :, 0] = pack_w(wq2 * ln1_g[l][:, None])
        wqkv[l, :, 1] = pack_w(wk2 * ln1_g[l][:, None])
        wqkv[l, :, 2] = pack_w(wv2 * ln1_g[l][:, None])
        biasr[l, :, 0:MT] = (ln1_b[l] @ wq2).reshape(MT, 128).T
        biasr[l, :, MT:2 * MT] = (ln1_b[l] @ wk2).reshape(MT, 128).T
        vbias[l] = np.broadcast_to(ln1_b[l] @ wv2, (128, D))
        wproj[l] = pack_w(Wproj[l])
        w1[l] = pack_w(W1[l] * ln2_g[l][:, None])
        w2[l] = pack_w(W2[l])
        biasc[l, :, 0:MT] = bproj[l].reshape(MT, 128).T
        biasc[l, :, MT:MT + FT] = (b1[l] + ln2_b[l] @ W1[l]).reshape(FT, 128).T
        biasc[l, :, MT + FT:] = b2[l].reshape(MT, 128).T

    whead_eff = Whead * lnf_g[:, None]
    bhead_eff = (bhead + lnf_b @ Whead).astype(f32)

    cst = np.ones((128, CST_W), NPBF)
    cst[:, 513:641] = np.eye(128, dtype=NPBF)
    cst[:, 641:769] = NPBF(1.0 / 512.0)

    lnc = np.zeros((128, 2), f32)
    lnc[:, 0] = EPS_EFF
    lnc[:, 1] = RSTD_BIAS

    return {
        "cst": cst,
        "lnc": lnc,
        "hotT": hot,
        "temb": tok_emb.astype(NPBF),
        "posT": pos_in,
        "mask": mask,
        "wqkv": wqkv,
        "wproj": wproj,
        "w1": w1,
        "w2": w2,
        "vbias": vbias,
        "biasc": biasc,
        "biasr": biasr,
        "whead": pack_w(whead_eff).astype(NPBF),
        "bheadc": bhead_eff[:, None].copy(),
    }


_CACHE = {}


def get_program():
    if "nc" not in _CACHE:
        _CACHE["nc"] = build_program()
    return _CACHE["nc"]


def run_on_hw(inputs, trace=False):
    nc = get_program()
    in_maps = [prep_inputs(inputs, core=c) for c in range(NCORES)]
    res = run_bass_kernel_spmd(nc, in_maps, list(range(NCORES)), trace=trace)
    outs = []
    for c in range(NCORES):
        lt = res.results[c]["logitsT"]          # [BL, V, T]
        outs.append(lt.transpose(0, 2, 1))      # [BL, T, V]
    full = np.concatenate(outs, axis=0)         # [B, T, V]
    return full, res


def kernel(**inputs):
    out, _ = run_on_hw(inputs, trace=False)
    return out
